# revision 7
# baseline (speedup 1.0000x reference)
"""Trainium2 Bass kernel for the NormalsRenderer problem.

Math: for each batch b (512 batches of N=512 normals, dim 3):
  s_ij = x_i . x_j ;  w_ij = exp(-arccos(clip(s_ij,-1,1))) (i != j)
  new_w_i = sum_j w_ij ;  n_b = sum_i new_w_i x_i ; output = n_b/||n_b||
(The reference's division by the global max of new_w cancels under the
final normalize, so the problem is purely batch-parallel.)

Device algorithm (per [128 x 512] tile of the pairwise matrix):
  exp(-arccos(clip(s))) ~= a0 + a1*c + g2*(c^2-1) + g3*(c^3-c) + u*Q(s)
where c = clip(s), u = sqrt(1-c^2) and Q is a degree-7 polynomial
(minimax fit, max abs err 5.7e-5; the c^3 basis kills the second-
derivative jumps of the sine series at the clip points).
  - s comes from a K=3 matmul on the TensorEngine
  - Q(s) comes from a K=120 matmul over monomial features of the
    vectors ((x_i.x_j)^k = <x_i^(k), x_j^(k)> tensor powers)
  - clip is a DVE dual-op tensor_scalar (for half the tiles, ACT
    computes relu(s+1) first so the DVE finish runs at 2x from SBUF)
  - c^2 (+ row-sum) and u are two ACT passes (Square, Sqrt)
  - the odd-basis row-sum uses one DVE scalar_tensor_tensor
    (c^2+lam)*c whose accumulator equals g3*sum(c^3) + (a1-g3)*sum(c)
  - u*Q (+ row-sum) is one DVE scalar_tensor_tensor (u+0)*q
Row sums accumulate per-tile into [128, 256] buffers; the diagonal's
contribution is computed separately on [128, 256] data and subtracted.
The final weighted sum of normals is 256 tiny K=128 matmuls.
"""

import numpy as np

import concourse.bass as bass
import concourse.mybir as mybir
from concourse.tile import TileContext
from concourse.bass_utils import run_bass_kernel_spmd

F32 = mybir.dt.float32
ALU = mybir.AluOpType
ACTF = mybir.ActivationFunctionType

# ----- approximation constants (minimax fit of exp(-arccos(x)) on [-1,1]) ---
E_PI = float(np.exp(-np.pi))
A0 = (1.0 + E_PI) / 2.0
A1 = (1.0 - E_PI) / 2.0
G2 = -0.24608868322823232
G3 = -0.44875064949785576
QCOEF = [
    -0.5598239146551482,
    -0.7197892496369586,
    0.07045122754007498,
    0.16558466625704432,
    0.00639389688770868,
    -0.0006506546739549137,
    0.0058886322787922015,
    0.034800521840699004,
]
QDEG = 7
LAM = (A1 - G3) / G3   # -2.066055372629382
CLIP_RELU_FRAC4 = 2   # of every 4 tiles, how many use the ACT-relu clip path

B_FULL, N, D = 512, 512, 3
N_CORES = 8
B_LOC = B_FULL // N_CORES          # 64 batches per core
N_CHUNK = N // 128                 # 4 i-chunks per batch
N_COLS = B_LOC * N_CHUNK           # 256 (layout columns for row-sum buffers)


def _monomial_exponents():
    exps = []
    for k in range(QDEG + 1):
        for ax in range(k, -1, -1):
            for ay in range(k - ax, -1, -1):
                az = k - ax - ay
                exps.append((ax, ay, az))
    return exps


_EXPS = _monomial_exponents()
N_FEAT = len(_EXPS)                # 120


def _multinom(a):
    from math import factorial
    k = sum(a)
    return factorial(k) // (factorial(a[0]) * factorial(a[1]) * factorial(a[2]))


def _features(xflat):
    """xflat: [V, 3] float32 -> (f_lhs [NF, V], f_rhs [NF, V]) float32."""
    V = xflat.shape[0]
    f_rhs = np.empty((N_FEAT, V), np.float32)
    f_lhs = np.empty((N_FEAT, V), np.float32)
    x0 = xflat[:, 0].astype(np.float64)
    x1 = xflat[:, 1].astype(np.float64)
    x2 = xflat[:, 2].astype(np.float64)
    for f, (ax, ay, az) in enumerate(_EXPS):
        mono = (x0 ** ax) * (x1 ** ay) * (x2 ** az)
        f_rhs[f] = mono.astype(np.float32)
        f_lhs[f] = (QCOEF[ax + ay + az] * _multinom((ax, ay, az)) * mono).astype(
            np.float32
        )
    return f_lhs, f_rhs


def _split_multiwaits(nc, max_waits=1):
    """walrus in this toolchain rejects >1 sem-wait on one instruction
    (setupSyncWait 'Too many sync wait commands'). Split extra waits into
    preceding single-wait EventSemaphore instructions on the same engine."""
    n_split = 0
    for func in nc.m.functions:
        for blk in func.blocks:
            insts = blk.instructions
            i = 0
            new_list = []
            for inst in insts:
                si = inst.sync_info
                if si is not None and si.on_wait and len(si.on_wait) > max_waits:
                    conds = list(si.on_wait)
                    for j, cond in enumerate(conds[:-max_waits]):
                        ev = mybir.InstEventSemaphore(
                            name=f"{inst.name}-xw{j}", ins=[], outs=[]
                        )
                        ev.engine = inst.engine
                        ev.sync_info = mybir.SyncInfo(on_wait=[cond], on_update=[])
                        new_list.append(ev)
                        n_split += 1
                    si.on_wait = conds[-max_waits:]
                new_list.append(inst)
                i += 1
            if n_split:
                while len(blk.instructions):
                    blk.instructions.pop()
                for inst in new_list:
                    blk.instructions.append(inst)
    return n_split


_NC_CACHE = []


def _build_program():
    if _NC_CACHE:
        return _NC_CACHE[0]
    nc = bass.Bass("TRN2", target_bir_lowering=False, debug=False,
                   num_devices=N_CORES)
    xt = nc.declare_dram_parameter("xt", [D, B_LOC * N], F32, isOutput=False)
    xnat = nc.declare_dram_parameter("xnat", [128, N_COLS * D], F32,
                                     isOutput=False)
    flhs = nc.declare_dram_parameter("flhs", [N_FEAT, B_LOC * N], F32,
                                     isOutput=False)
    frhs = nc.declare_dram_parameter("frhs", [N_FEAT, B_LOC * N], F32,
                                     isOutput=False)
    nout = nc.declare_dram_parameter("nout", [1, B_LOC * D], F32, isOutput=True)

    with TileContext(nc) as tc:
        with (
            tc.tile_pool(name="resident", bufs=1) as res_pool,
            tc.tile_pool(name="flhs_pool", bufs=3) as flhs_pool,
            tc.tile_pool(name="frhs_pool", bufs=3) as frhs_pool,
            tc.tile_pool(name="work", bufs=3) as work,
            tc.tile_pool(name="small", bufs=1) as small,
            tc.tile_pool(name="spsum", bufs=2, space="PSUM") as spsum,
            tc.tile_pool(name="qpsum", bufs=2, space="PSUM") as qpsum,
            tc.tile_pool(name="npsum", bufs=1, space="PSUM") as npsum,
        ):
            xt_sb = res_pool.tile([D, B_LOC * N], F32)
            nc.sync.dma_start(xt_sb[:], xt[:])
            xnat_sb = res_pool.tile([128, N_COLS * D], F32)
            nc.sync.dma_start(xnat_sb[:], xnat[:])

            # row-sum accumulator buffers, one column per (batch, chunk)
            rc2 = res_pool.tile([128, N_COLS], F32)
            rc3 = res_pool.tile([128, N_COLS], F32)
            ruq = res_pool.tile([128, N_COLS], F32)

            # ---------------- diagonal correction (tiny, [128, 256]) -------
            xsq = small.tile([128, N_COLS * D], F32)
            nc.scalar.activation(xsq[:], xnat_sb[:], ACTF.Square, bias=0.0,
                                 scale=1.0)
            sd = small.tile([128, N_COLS], F32)
            nc.vector.tensor_reduce(
                sd[:], xsq.rearrange("p (v d) -> p v d", d=D),
                axis=mybir.AxisListType.X, op=ALU.add)
            cd = small.tile([128, N_COLS], F32)
            nc.vector.tensor_scalar(cd[:], sd[:], 1.0, None, ALU.min)
            hd = small.tile([128, N_COLS], F32)
            nc.scalar.activation(hd[:], cd[:], ACTF.Square, bias=0.0, scale=1.0)
            ud = small.tile([128, N_COLS], F32)
            nc.scalar.activation(ud[:], hd[:], ACTF.Sqrt, bias=1.0, scale=-1.0)
            # Q(sd) by Horner (mult + add pairs)
            qd = small.tile([128, N_COLS], F32)
            nc.vector.memset(qd[:], QCOEF[QDEG])
            tmp = small.tile([128, N_COLS], F32)
            for k in range(QDEG - 1, -1, -1):
                nc.vector.tensor_tensor(tmp[:], qd[:], sd[:], ALU.mult)
                nc.vector.tensor_scalar(qd[:], tmp[:], float(QCOEF[k]), None,
                                        ALU.add)
            # tau_d = a0 + a1*cd + g2*(hd-1) + g3*(hd-1)*cd + ud*qd
            v2 = small.tile([128, N_COLS], F32)
            nc.vector.scalar_tensor_tensor(v2[:], hd[:], -1.0, cd[:], ALU.add,
                                           ALU.mult)
            taud = small.tile([128, N_COLS], F32)
            nc.vector.tensor_scalar(taud[:], hd[:], G2, A0 - G2, ALU.mult,
                                    ALU.add)
            nc.vector.scalar_tensor_tensor(taud[:], cd[:], A1, taud[:],
                                           ALU.mult, ALU.add)
            nc.vector.scalar_tensor_tensor(taud[:], v2[:], G3, taud[:],
                                           ALU.mult, ALU.add)
            uqd = small.tile([128, N_COLS], F32)
            nc.vector.tensor_tensor(uqd[:], ud[:], qd[:], ALU.mult)
            nc.vector.tensor_tensor(taud[:], taud[:], uqd[:], ALU.add)

            # ---------------- main loop over (batch, chunk) -----------------
            for b in range(B_LOC):
                frhs_t = frhs_pool.tile([N_FEAT, N], F32)
                nc.sync.dma_start(frhs_t[:], frhs[:, b * N:(b + 1) * N])
                for m in range(N_CHUNK):
                    col = b * N_CHUNK + m
                    off = b * N + m * 128
                    flhs_t = flhs_pool.tile([N_FEAT, 128], F32)
                    nc.sync.dma_start(flhs_t[:], flhs[:, off:off + 128])

                    s_ps = spsum.tile([128, N], F32)
                    nc.tensor.matmul(s_ps[:], xt_sb[:, off:off + 128],
                                     xt_sb[:, b * N:(b + 1) * N],
                                     start=True, stop=True)
                    q_ps = qpsum.tile([128, N], F32)
                    nc.tensor.matmul(q_ps[:], flhs_t[:], frhs_t[:],
                                     start=True, stop=True)

                    c_t = work.tile([128, N], F32, tag="c_t")
                    if (col % 4) < CLIP_RELU_FRAC4:
                        # rebalance: ACT computes (s+1)+, DVE finishes the
                        # clip from SBUF at 2x rate: c = min(relu(s+1)-1, 1)
                        r_t = work.tile([128, N], F32, tag="r_t")
                        nc.scalar.activation(r_t[:], s_ps[:], ACTF.Relu,
                                             bias=1.0, scale=1.0)
                        nc.vector.tensor_scalar(
                            c_t[:], r_t[:], -1.0, 1.0, ALU.add, ALU.min)
                    else:
                        nc.vector.tensor_scalar(
                            c_t[:], s_ps[:], -1.0, 1.0, ALU.max, ALU.min)
                    h_t = work.tile([128, N], F32, tag="h_t")
                    nc.scalar.activation(h_t[:], c_t[:], ACTF.Square, bias=0.0,
                                         scale=1.0,
                                         accum_out=rc2[:, col:col + 1])
                    u_t = work.tile([128, N], F32, tag="u_t")
                    nc.scalar.activation(u_t[:], h_t[:], ACTF.Sqrt, bias=1.0,
                                         scale=-1.0)
                    t3 = work.tile([128, N], F32, tag="t3")
                    nc.vector.scalar_tensor_tensor(
                        t3[:], h_t[:], LAM, c_t[:], ALU.add, ALU.mult,
                        accum_out=rc3[:, col:col + 1])
                    uq = work.tile([128, N], F32, tag="uq")
                    nc.vector.scalar_tensor_tensor(
                        uq[:], u_t[:], 0.0, q_ps[:], ALU.add, ALU.mult,
                        accum_out=ruq[:, col:col + 1])

            # ---------------- combine row sums ------------------------------
            # W = 512*(a0-g2) + g2*rc2 + g3*rc3' + ruq - taud
            #   where rc3' = sum((c^2+LAM)*c) = sum(c^3) + (a1-g3)/g3*sum(c)
            w_t = small.tile([128, N_COLS], F32)
            nc.vector.tensor_scalar(w_t[:], rc2[:], G2,
                                    float(N) * (A0 - G2), ALU.mult, ALU.add)
            nc.vector.scalar_tensor_tensor(w_t[:], rc3[:], G3, w_t[:],
                                           ALU.mult, ALU.add)
            nc.vector.tensor_tensor(w_t[:], w_t[:], ruq[:], ALU.add)
            nc.vector.tensor_tensor(w_t[:], w_t[:], taud[:], ALU.subtract)

            # ---------------- weighted sum of normals -----------------------
            n_ps = npsum.tile([1, B_LOC * D], F32)
            for b in range(B_LOC):
                for m in range(N_CHUNK):
                    col = b * N_CHUNK + m
                    nc.tensor.matmul(
                        n_ps[0:1, b * D:(b + 1) * D],
                        w_t[:, col:col + 1],
                        xnat_sb[:, col * D:col * D + D],
                        start=(m == 0), stop=(m == N_CHUNK - 1))
            n_sb = small.tile([1, B_LOC * D], F32)
            nc.vector.tensor_copy(n_sb[:], n_ps[:])
            nc.sync.dma_start(nout[:], n_sb[:])

    _split_multiwaits(nc)
    _NC_CACHE.append(nc)
    return nc


def kernel(normals, weights=None):
    normals = np.ascontiguousarray(normals, dtype=np.float32)
    assert normals.shape == (B_FULL, N, D)

    nc = _build_program()

    in_maps = []
    for core in range(N_CORES):
        xc = normals[core * B_LOC:(core + 1) * B_LOC]          # [64, 512, 3]
        xflat = xc.reshape(B_LOC * N, D)                        # [32768, 3]
        xt = np.ascontiguousarray(xflat.T)                      # [3, 32768]
        xnat = np.ascontiguousarray(
            xc.reshape(B_LOC, N_CHUNK, 128, D).transpose(2, 0, 1, 3)
            .reshape(128, N_COLS * D))
        f_lhs, f_rhs = _features(xflat)
        in_maps.append({"xt": xt, "xnat": xnat, "flhs": f_lhs, "frhs": f_rhs})

    res = run_bass_kernel_spmd(nc, in_maps, core_ids=list(range(N_CORES)))

    v = np.concatenate(
        [res.results[i]["nout"].reshape(B_LOC, D) for i in range(N_CORES)], 0)
    out = v / np.sqrt(np.maximum((v * v).sum(-1, keepdims=True), 1e-20))
    return out.astype(np.float32)


if __name__ == "__main__":
    x = np.random.RandomState(0).randn(B_FULL, N, D).astype(np.float32)
    print(kernel(x)[:4])
